# revision 1
# baseline (speedup 1.0000x reference)
"""Trainium2 Bass kernel for nn_DeformConv2d (modulated deformable conv).

Strategy (data-parallel over batch, one batch element per NeuronCore):
  1. Offset conv (grouped, dil=2) as 9 accumulating PE matmuls over a padded
     image, block-diagonal tap weights -> om [54, 4096].
  2. PE-transpose om into sample-major layout; compute sampling coordinates,
     bilinear corner coefficients (mask & zero-pad validity folded in) and
     gather row indices with fat DVE ops.
  3. Per (group, tap): indirect-DMA gather of 2x2 corner pixel pairs from a
     guard-padded DRAM pixel-row table (out-of-bounds samples land in zero
     guard rows), then 4 per-partition-scalar DVE ops blend the corners.
  4. PE-transpose blended samples to channel-major and run the main
     contraction as 9 accumulating float32r matmuls -> out [128, 4096].
"""

import numpy as np

import concourse.bass as bass
import concourse.tile as tile
from concourse import bacc, mybir
from concourse.bass_utils import run_bass_kernel_spmd
from concourse.masks import make_identity

# Problem constants (hardcoded per the harness contract).
B, C, H, W, Co = 8, 128, 64, 64, 128
KS, DIL, PAD, DG = 3, 2, 2, 2
KK = KS * KS          # 9
Cg = C // DG          # 64
NO = DG * 3 * KK      # 54 offset-conv output channels
NOFF = DG * 2 * KK    # 36 offset channels
S = H * W             # 4096 output pixels
HP = H + 2 * PAD      # 68 padded conv image side
GB = 6                # guard border for the gather row table
GY = W + 2 * GB       # 76 guarded row width
NR = GY * GY          # 5776 pixel rows in guard layout
NJ = DG * KK          # 18 (g,k) pairs
NT = 32               # 4096 / 128 sample tiles
F32 = mybir.dt.float32
F32R = mybir.dt.float32r
I32 = mybir.dt.int32
AL = mybir.AluOpType
ACTF = mybir.ActivationFunctionType

# Index arithmetic for the guard layout: pixel (y, x) lives at row
# (y+GB)*GY + (x+GB); r_top = y0*GY + x0 + IDX_OFF.
IDX_OFF = GB * GY + GB  # 462

USE_F32R = True
MMDT = F32R if USE_F32R else F32


def _mmdt(ap):
    return ap


def build_nc(debug_taps=False):
    nc = bacc.Bacc(None)
    dbg = {}
    def tap(name, shape, dt_=F32):
        if debug_taps:
            dbg[name] = nc.dram_tensor("dbg_" + name, shape, dt_,
                                       kind="ExternalOutput")
        return dbg.get(name)

    xpad = nc.dram_tensor("xpad", [C, HP * HP], MMDT, kind="ExternalInput")
    xpr0 = nc.dram_tensor("xpr0", [NR, 4 * Cg], F32, kind="ExternalInput")
    xpr1 = nc.dram_tensor("xpr1", [NR, 4 * Cg], F32, kind="ExternalInput")
    offw = nc.dram_tensor("offw", [KK, C, NO], MMDT, kind="ExternalInput")
    offb = nc.dram_tensor("offb", [NO, 1], F32, kind="ExternalInput")
    wmain = nc.dram_tensor("wmain", [KK, C, Co], MMDT, kind="ExternalInput")
    byx = nc.dram_tensor("byx", [2, 128, NT * NJ], F32, kind="ExternalInput")
    out = nc.dram_tensor("out", [Co, S], F32, kind="ExternalOutput")
    xprs = [xpr0, xpr1]

    with tile.TileContext(nc) as tc:
        with (
            tc.tile_pool(name="const", bufs=1) as cpool,
            tc.tile_pool(name="fields", bufs=1) as fpool,
        ):
            ident = cpool.tile([128, 128], F32)
            make_identity(nc, ident[:, :])

            ow_sb = cpool.tile([128, KK, NO], MMDT)
            nc.sync.dma_start(ow_sb[:, :, :], offw.rearrange("k c o -> c k o"))
            ob_sb = cpool.tile([NO, 1], F32)
            nc.sync.dma_start(ob_sb[:, :], offb[:, :])
            wm_sb = cpool.tile([128, KK, Co], MMDT)
            nc.sync.dma_start(wm_sb[:, :, :], wmain.rearrange("k c o -> c k o"))
            by_sb = cpool.tile([128, NT * NJ], F32)
            nc.sync.dma_start(by_sb[:, :], byx[0])
            bx_sb = cpool.tile([128, NT * NJ], F32)
            nc.sync.dma_start(bx_sb[:, :], byx[1])

            # ---- Phase B: offset conv -> om_sb [54, 4096] ----------------
            om_sb = fpool.tile([NO, S], F32)
            omT = fpool.tile([128, NT * NO], F32)
            with (
                tc.tile_pool(name="xpad", bufs=1) as xpool,
                tc.tile_pool(name="psBC", bufs=2, space="PSUM") as psBC,
            ):
                xp_sb = xpool.tile([C, HP * HP], MMDT)
                nc.sync.dma_start(xp_sb[:, :], xpad[:, :])
                xp3 = xp_sb.rearrange("c (r q) -> c r q", q=HP)
                for ch in range(8):  # 8 chunks of 512 output pixels
                    om_ps = psBC.tile([NO, 512], F32, tag="omps", name="omps")
                    for k in range(KK):
                        ky, kx = k // KS, k % KS
                        rhs = xp3[:, 2 * ky + ch * 8 : 2 * ky + ch * 8 + 8,
                                  2 * kx : 2 * kx + W]
                        nc.tensor.matmul(
                            om_ps[:, :], _mmdt(ow_sb[:, k, :]), _mmdt(rhs),
                            start=(k == 0), stop=(k == KK - 1),
                        )
                    nc.scalar.activation(
                        om_sb[:, ch * 512 : (ch + 1) * 512], om_ps[:, :],
                        ACTF.Identity, bias=ob_sb[:, :], scale=1.0,
                    )

                # ---- Phase C: transpose om -> omT [128, 32*54] -----------
                for n in range(NT):
                    tp = psBC.tile([128, NO], F32, tag="omt", name="omt")
                    nc.tensor.transpose(
                        tp[:, :], om_sb[:, n * 128 : (n + 1) * 128],
                        ident[:NO, :NO]
                    )
                    nc.scalar.copy(omT[:, n * NO : (n + 1) * NO], tp[:, :])

            t_om = tap("om", [NO, S])
            if t_om is not None:
                nc.sync.dma_start(t_om[:, :], om_sb[:, :])
            t_omT = tap("omT", [128, NT * NO])
            if t_omT is not None:
                nc.sync.dma_start(t_omT[:, :], omT[:, :])

            # ---- Phase D: coordinates, coefficients, indices --------------
            omT3 = omT.rearrange("p (n c) -> p n c", c=NO)
            # offset slices as [p, n, g, k] views (yx major split last)
            offv = omT3[:, :, 0:NOFF].rearrange("p n (g k t) -> p n g k t", g=DG, k=KK)
            maskv = omT3[:, :, NOFF:NO].rearrange("p n (g k) -> p n g k", g=DG)

            def F(nm):
                return fpool.tile([128, NT * NJ], F32, name=nm)

            def v4(t):  # [128, 576] -> [p, n, g, k] view (j-major layout)
                return t.rearrange("p (g k n) -> p n g k", g=DG, k=KK)

            py, px = F("py"), F("px")
            nc.vector.tensor_tensor(out=v4(py), in0=offv[:, :, :, :, 0],
                                    in1=v4(by_sb), op=AL.add)
            nc.vector.tensor_tensor(out=v4(px), in0=offv[:, :, :, :, 1],
                                    in1=v4(bx_sb), op=AL.add)
            for t_ in (py, px):
                nc.vector.tensor_scalar_max(t_[:, :], t_[:, :], -5.5)
                nc.vector.tensor_scalar_min(t_[:, :], t_[:, :], 67.5)

            def floor_of(src, nm):
                fl = F("fl_" + nm)
                ii = fpool.tile([128, NT * NJ], I32, name="ii_" + nm)
                nc.vector.tensor_scalar_add(fl[:, :], src[:, :], 1024.0)
                nc.vector.tensor_copy(out=ii[:, :], in_=fl[:, :])
                nc.vector.tensor_copy(out=fl[:, :], in_=ii[:, :])
                nc.vector.tensor_scalar_sub(fl[:, :], fl[:, :], 1024.0)
                fix = F("fix_" + nm)
                nc.vector.tensor_tensor(out=fix[:, :], in0=fl[:, :], in1=src[:, :],
                                        op=AL.is_gt)
                nc.vector.tensor_tensor(out=fl[:, :], in0=fl[:, :], in1=fix[:, :],
                                        op=AL.subtract)
                return fl

            y0, x0 = floor_of(py, "y"), floor_of(px, "x")
            wy, wx = F("wy"), F("wx")
            nc.vector.tensor_tensor(out=wy[:, :], in0=py[:, :], in1=y0[:, :],
                                    op=AL.subtract)
            nc.vector.tensor_tensor(out=wx[:, :], in0=px[:, :], in1=x0[:, :],
                                    op=AL.subtract)

            mm = F("mm")
            nc.scalar.activation(v4(mm), maskv, ACTF.Sigmoid)
            nc.vector.tensor_scalar_mul(mm[:, :], mm[:, :], 2.0)

            beta, alpha = F("beta"), F("alpha")
            nc.vector.tensor_tensor(out=beta[:, :], in0=mm[:, :], in1=wy[:, :],
                                    op=AL.mult)
            nc.vector.tensor_tensor(out=alpha[:, :], in0=mm[:, :], in1=beta[:, :],
                                    op=AL.subtract)
            c01, c00 = F("c01"), F("c00")
            nc.vector.tensor_tensor(out=c01[:, :], in0=alpha[:, :], in1=wx[:, :],
                                    op=AL.mult)
            nc.vector.tensor_tensor(out=c00[:, :], in0=alpha[:, :], in1=c01[:, :],
                                    op=AL.subtract)
            c11, c10 = F("c11"), F("c10")
            nc.vector.tensor_tensor(out=c11[:, :], in0=beta[:, :], in1=wx[:, :],
                                    op=AL.mult)
            nc.vector.tensor_tensor(out=c10[:, :], in0=beta[:, :], in1=c11[:, :],
                                    op=AL.subtract)

            itf = F("itf")
            nc.vector.tensor_scalar(itf[:, :], y0[:, :], float(GY),
                                    float(IDX_OFF), AL.mult, AL.add)
            nc.vector.tensor_tensor(out=itf[:, :], in0=itf[:, :], in1=x0[:, :],
                                    op=AL.add)
            it_i = fpool.tile([128, NT * NJ], I32, name="it_i")
            nc.vector.tensor_copy(out=it_i[:, :], in_=itf[:, :])
            coefs = [c00, c01, c10, c11]
            for nm_, t_ in (("c00", c00), ("c01", c01), ("c10", c10),
                            ("c11", c11), ("wy", wy), ("wx", wx)):
                tt = tap(nm_, [128, NT * NJ])
                if tt is not None:
                    nc.sync.dma_start(tt[:, :], t_[:, :])
            t_it = tap("it", [128, NT * NJ], I32)
            if t_it is not None:
                nc.sync.dma_start(t_it[:, :], it_i[:, :])

            # ---- Phase E/F: gather, blend, transpose, main matmul ---------
            from contextlib import ExitStack
            ectx = ExitStack()
            gpool = ectx.enter_context(tc.tile_pool(name="gather", bufs=3))
            vpool = ectx.enter_context(tc.tile_pool(name="vpairp", bufs=2))
            vtpool = ectx.enter_context(tc.tile_pool(name="valtp", bufs=2))
            opool = ectx.enter_context(tc.tile_pool(name="outsbp", bufs=2))
            psO = ectx.enter_context(tc.tile_pool(name="psO", bufs=1, space="PSUM"))
            psT = ectx.enter_context(tc.tile_pool(name="psT", bufs=4, space="PSUM"))
            for half in range(2):
                out_ps = psO.tile([128, 2048], F32, tag="out", name="out_ps")
                n0 = half * 16
                for k in range(KK):
                    vpair = vpool.tile([128, 16, 128], F32, tag="vp", name="vpair")
                    for g in range(DG):
                        j = g * KK + k
                        gt = gpool.tile([128, 16, 256], F32, tag="gt", name="gt")
                        for n in range(16):
                            ic = j * NT + n0 + n
                            nc.gpsimd.indirect_dma_start(
                                out=gt[:, n, :],
                                out_offset=None,
                                in_=xprs[g][:, :],
                                in_offset=bass.IndirectOffsetOnAxis(
                                    ap=it_i[:, ic : ic + 1], axis=0,
                                ),
                            )
                        if half == 0 and k == 0 and g == 0:
                            t_gt = tap("gt00", [128, 16, 256])
                            if t_gt is not None:
                                nc.sync.dma_start(t_gt[:, :, :], gt[:, :, :])
                        for n in range(16):
                            col = j * NT + (n0 + n)
                            vslice = vpair[:, n, g * Cg : (g + 1) * Cg]
                            srcs = (gt[:, n, 0:Cg], gt[:, n, Cg:2 * Cg],
                                    gt[:, n, 2 * Cg:3 * Cg],
                                    gt[:, n, 3 * Cg:4 * Cg])
                            nc.vector.tensor_scalar_mul(
                                vslice, srcs[0], coefs[0][:, col : col + 1])
                            for ci in range(1, 4):
                                nc.vector.scalar_tensor_tensor(
                                    out=vslice, in0=srcs[ci],
                                    scalar=coefs[ci][:, col : col + 1],
                                    in1=vslice, op0=AL.mult, op1=AL.add)
                    if half == 0 and k == 0:
                        t_vp = tap("vp00", [128, 16, 128])
                        if t_vp is not None:
                            nc.sync.dma_start(t_vp[:, :, :], vpair[:, :, :])
                    valT = vtpool.tile([128, 2048], MMDT, tag="vt", name="valT")
                    for n in range(16):
                        tp = psT.tile([128, 128], F32, tag="vtp", name="tp_v")
                        nc.tensor.transpose(tp[:, :], vpair[:, n, :], ident[:, :])
                        nc.scalar.copy(valT[:, n * 128 : (n + 1) * 128], tp[:, :])
                    if half == 0 and k == 0:
                        t_vt = tap("valT00", [128, 2048])
                        if t_vt is not None:
                            nc.sync.dma_start(t_vt[:, :], valT[:, :].bitcast(F32))
                    for jc in range(4):
                        cs = slice(jc * 512, (jc + 1) * 512)
                        nc.tensor.matmul(
                            out_ps[:, cs], _mmdt(wm_sb[:, k, :]), _mmdt(valT[:, cs]),
                            start=(k == 0), stop=(k == KK - 1),
                        )
                o_sb = opool.tile([128, 2048], F32, tag="osb", name="o_sb")
                for jc in range(4):
                    cs = slice(jc * 512, (jc + 1) * 512)
                    nc.scalar.copy(o_sb[:, cs], out_ps[:, cs])
                nc.sync.dma_start(out[:, half * 2048 : (half + 1) * 2048],
                                  o_sb[:, :])
            ectx.close()
    nc.finalize()
    return nc


def host_inputs(x, offset_w, offset_b, weight):
    """Build the per-core input maps (core b <- batch element b)."""
    x = np.asarray(x, np.float32)
    offset_w = np.asarray(offset_w, np.float32)
    offset_b = np.asarray(offset_b, np.float32)
    weight = np.asarray(weight, np.float32)

    # Tap weights, block-diagonal over conv groups: [KK, C, NO]
    offw = np.zeros((KK, C, NO), np.float32)
    for k in range(KK):
        ky, kx = k // KS, k % KS
        for g in range(DG):
            # conv group g: out chans [g*27,(g+1)*27) <- in chans [g*64,(g+1)*64)
            offw[k, g * Cg:(g + 1) * Cg, g * 27:(g + 1) * 27] = \
                offset_w[g * 27:(g + 1) * 27, :, ky, kx].T
    offb = offset_b.reshape(NO, 1).copy()

    # Main weights: [KK, C, Co] with rows (g*64+c) = weight[o, g*64+c, ky, kx]
    wmain = np.zeros((KK, C, Co), np.float32)
    for k in range(KK):
        ky, kx = k // KS, k % KS
        wmain[k] = weight[:, :, ky, kx].T  # [C, Co]

    # Base grid constants, j-major layout: col = (g*9+k)*32 + n
    p_idx = np.arange(128)
    n_idx = np.arange(NT)
    s = n_idx[None, :] * 128 + p_idx[:, None]          # [128, 32]
    hh, ww = s // W, s % W
    by = np.zeros((128, NJ, NT), np.float32)
    bx = np.zeros((128, NJ, NT), np.float32)
    for g in range(DG):
        for k in range(KK):
            ky, kx = k // KS, k % KS
            by[:, g * KK + k, :] = hh + 2 * ky - 2
            bx[:, g * KK + k, :] = ww + 2 * kx - 2
    byx = np.stack([by.reshape(128, NJ * NT), bx.reshape(128, NJ * NT)])

    in_maps = []
    for b in range(B):
        xb = x[b]  # [C, H, W]
        xpad = np.zeros((C, HP, HP), np.float32)
        xpad[:, PAD:PAD + H, PAD:PAD + W] = xb
        # Guarded pixel-row tables per sampling group, with pair rows.
        maps = {
            "xpad": xpad.reshape(C, HP * HP),
            "offw": offw, "offb": offb, "wmain": wmain, "byx": byx,
        }
        for g in range(DG):
            grows = np.zeros((GY, GY, Cg), np.float32)
            grows[GB:GB + H, GB:GB + W, :] = \
                xb[g * Cg:(g + 1) * Cg].transpose(1, 2, 0)
            flat = np.concatenate(
                [grows.reshape(NR * Cg), np.zeros((GY + 2) * Cg, np.float32)])
            A = np.lib.stride_tricks.as_strided(
                flat, shape=(NR + GY + 1, 2 * Cg), strides=(Cg * 4, 4))
            patch = np.concatenate([A[:NR], A[GY:GY + NR]], axis=1).copy()
            maps[f"xpr{g}"] = patch
        in_maps.append(maps)
    return in_maps


_NC_CACHE = {}


def get_nc():
    if "nc" not in _NC_CACHE:
        _NC_CACHE["nc"] = build_nc()
    return _NC_CACHE["nc"]


def kernel(x, offset_w, offset_b, weight):
    nc = get_nc()
    in_maps = host_inputs(x, offset_w, offset_b, weight)
    res = run_bass_kernel_spmd(nc, in_maps, list(range(B)))
    outs = [res.results[b]["out"].reshape(Co, H, W) for b in range(B)]
    return np.stack(outs).astype(np.float32)



# revision 9
# speedup vs baseline: 3.8643x; 3.8643x over previous
"""Trainium2 Bass kernel for nn_DeformConv2d (modulated deformable conv).

Strategy (data-parallel over batch, one batch element per NeuronCore).
The axon tunnel moves ~40 MB/s, so wall-clock is dominated by host<->device
bytes: ship only x (bf16) + weights (bf16) per core and build everything
else on device:
  1. PE-transpose x to pixel-major and scatter it into guard-padded DRAM
     gather-row tables (4 bilinear corner pixel blocks per row, zero guard
     rows for out-of-bounds samples). Build the zero-padded conv image in
     SBUF directly.
  2. Offset conv (grouped, dil=2) as 9 accumulating PE matmuls with
     block-diagonal tap weights -> om [54, 4096].
  3. PE-transpose om into sample-major layout; compute sampling coords,
     bilinear corner coefficients (mask folded in) and gather row indices
     with fat DVE ops; base grid comes from on-device iota.
  4. Per (group, tap): indirect-DMA gather of the 4-corner rows, 4
     per-partition-scalar DVE ops blend the corners.
  5. PE-transpose blended samples to channel-major; main contraction as 9
     accumulating bf16 matmuls -> out [128, 4096] bf16.
"""

import numpy as np

import concourse.bass as bass
import concourse.tile as tile
from concourse import bacc, mybir
from concourse.bass_utils import run_bass_kernel_spmd
from concourse.masks import make_identity

# Problem constants (hardcoded per the harness contract).
B, C, H, W, Co = 8, 128, 64, 64, 128
KS, DIL, PAD, DG = 3, 2, 2, 2
KK = KS * KS          # 9
Cg = C // DG          # 64
NO = DG * 3 * KK      # 54 offset-conv output channels
NOFF = DG * 2 * KK    # 36 offset channels
S = H * W             # 4096 output pixels
HP = H + 2 * PAD      # 68 padded conv image side
GB = 6                # guard border for the gather row table
GY = W + 2 * GB       # 76 guarded row width
NR = GY * GY          # 5776 pixel rows in guard layout
NJ = DG * KK          # 18 (g,k) pairs
NT = 32               # 4096 / 128 sample tiles
F32 = mybir.dt.float32
BF16 = mybir.dt.bfloat16
I32 = mybir.dt.int32
AL = mybir.AluOpType
ACTF = mybir.ActivationFunctionType
NPBF = mybir.dt.np(BF16)

# Index arithmetic for the guard layout: pixel (y, x) lives at row
# (y+GB)*GY + (x+GB); r_top = y0*GY + x0 + IDX_OFF.
IDX_OFF = GB * GY + GB  # 462


def build_nc(debug_taps=False):
    nc = bacc.Bacc(None)
    dbg = {}

    def tap(name, shape, dt_=F32):
        if debug_taps:
            dbg[name] = nc.dram_tensor("dbg_" + name, shape, dt_,
                                       kind="ExternalOutput")
        return dbg.get(name)

    x_in = nc.dram_tensor("x_in", [C, S], BF16, kind="ExternalInput")
    offw = nc.dram_tensor("offw", [KK, C, NO], BF16, kind="ExternalInput")
    offb = nc.dram_tensor("offb", [NO, 1], F32, kind="ExternalInput")
    wmain = nc.dram_tensor("wmain", [KK, C, Co], BF16, kind="ExternalInput")
    out = nc.dram_tensor("out", [Co, S], BF16, kind="ExternalOutput")

    with tile.TileContext(nc) as tc:
        with (
            tc.tile_pool(name="const", bufs=1) as cpool,
            tc.tile_pool(name="fields", bufs=1) as fpool,
            tc.tile_pool(name="dram", bufs=1, space="DRAM") as dpool,
        ):
            ident = cpool.tile([128, 128], F32)
            make_identity(nc, ident[:, :])
            ident_bf = cpool.tile([128, 128], BF16)
            make_identity(nc, ident_bf[:, :])

            ow_sb = cpool.tile([128, KK, NO], BF16)
            nc.sync.dma_start(ow_sb[:, :, :], offw.rearrange("k c o -> c k o"))
            ob_sb = cpool.tile([NO, 1], F32)
            nc.sync.dma_start(ob_sb[:, :], offb[:, :])
            wm_sb = cpool.tile([128, KK, Co], BF16)
            nc.sync.dma_start(wm_sb[:, :, :], wmain.rearrange("k c o -> c k o"))

            # Base sampling grid, j-major layout: col = (g*9+k)*32 + n.
            # by[p, col] = 2*n + p//64 + 2*ky - 2 ; bx[p, col] = p%64 + 2*kx - 2
            pidx = cpool.tile([128, 1], I32)
            nc.gpsimd.iota(pidx[:, :], [[0, 1]], base=0, channel_multiplier=1)
            pf = cpool.tile([128, 1], F32)
            nc.vector.tensor_copy(out=pf[:, :], in_=pidx[:, :])
            p64 = cpool.tile([128, 1], F32)
            nc.vector.tensor_scalar(p64[:, :], pf[:, :], 64.0, None, AL.is_ge)
            pm64 = cpool.tile([128, 1], F32)
            nc.vector.scalar_tensor_tensor(out=pm64[:, :], in0=p64[:, :],
                                           scalar=-64.0, in1=pf[:, :],
                                           op0=AL.mult, op1=AL.add)
            byi = cpool.tile([128, NT * NJ], I32)
            nc.gpsimd.iota(byi[:, :], [[0, DG], [2, KS], [0, KS], [2, NT]],
                           base=-2, channel_multiplier=0)
            bxi = cpool.tile([128, NT * NJ], I32)
            nc.gpsimd.iota(bxi[:, :], [[0, DG], [0, KS], [2, KS], [0, NT]],
                           base=-2, channel_multiplier=0)
            by_sb = cpool.tile([128, NT * NJ], F32)
            nc.vector.tensor_copy(out=by_sb[:, :], in_=byi[:, :])
            nc.vector.tensor_scalar_add(by_sb[:, :], by_sb[:, :], p64[:, 0:1])
            bx_sb = cpool.tile([128, NT * NJ], F32)
            nc.vector.tensor_copy(out=bx_sb[:, :], in_=bxi[:, :])
            nc.vector.tensor_scalar_add(bx_sb[:, :], bx_sb[:, :], pm64[:, 0:1])

            # Guard-padded 4-corner gather tables, one per sampling group.
            xprs = [dpool.tile([NR, 4 * Cg], BF16, name=f"xpr{g}")
                    for g in range(DG)]

            # ---- Phase A: build gather tables + padded conv image ---------
            om_sb = fpool.tile([NO, S], F32)
            omT = fpool.tile([128, NT * NO], F32)
            xp_sb = fpool.tile([C, HP * HP], BF16)
            with (
                tc.tile_pool(name="build", bufs=1) as bpool,
                tc.tile_pool(name="psA", bufs=2, space="PSUM") as psA,
            ):
                x_sb = bpool.tile([C, S], BF16)
                nc.sync.dma_start(x_sb[:, :], x_in[:, :])

                # padded conv image in SBUF: zero border + interior copy
                nc.vector.memset(xp_sb[:, :], 0.0)
                xp3 = xp_sb.rearrange("c (r q) -> c r q", q=HP)
                nc.vector.tensor_copy(
                    out=xp3[:, PAD:PAD + H, PAD:PAD + W],
                    in_=x_sb.rearrange("c (h w) -> c h w", w=W))

                # pixel-major x: xt[p, n*128+c] = x[c, n*128+p]
                xt_sb = bpool.tile([128, S], BF16)
                for n in range(NT):
                    tp = psA.tile([128, 128], BF16, tag="tpx", name="tp_x")
                    nc.tensor.transpose(tp[:, :], x_sb[:, n * 128:(n + 1) * 128],
                                        ident_bf[:, :])
                    nc.scalar.copy(xt_sb[:, n * 128:(n + 1) * 128], tp[:, :])

                z_sb = bpool.tile([128, 6 * 256], BF16)
                nc.vector.memset(z_sb[:, :], 0.0)
                for g in range(DG):
                    r2 = xprs[g]
                    # zero the guard rows (top/bottom bands + side strips)
                    for a in range(0, 384, 128):
                        nc.sync.dma_start(r2[a:a + 128, :], z_sb[:, 0:256])
                        nc.sync.dma_start(r2[NR - 456 + a:NR - 328 + a, :],
                                          z_sb[:, 0:256])
                    nc.sync.dma_start(r2[384:456, :], z_sb[0:72, 0:256])
                    nc.sync.dma_start(r2[NR - 72:NR, :], z_sb[0:72, 0:256])
                    v2 = r2.rearrange("(a b) w -> a b w", b=GY)
                    nc.sync.dma_start(
                        v2[GB:GB + H, 0:GB, :],
                        z_sb[0:64, :].rearrange("p (b w) -> p b w", w=256))
                    nc.sync.dma_start(
                        v2[GB:GB + H, GB + W:GY, :],
                        z_sb[0:64, :].rearrange("p (b w) -> p b w", w=256))
                    # interior far edge (a=69 / b=69): shifted corners need
                    # zeros there; corner 0 overwrites its own block after.
                    nc.sync.dma_start(v2[GB + H - 1, :, :],
                                      z_sb[0:GY, 0:256])
                    nc.sync.dma_start(v2[GB:GB + H, GB + W - 1, :],
                                      z_sb[0:64, 0:256])
                    # corner blocks: table(a, b, j) = pix(a+dy, b+dx)
                    v4 = r2.rearrange("(a b) (j c) -> a b j c", b=GY, c=Cg)
                    for j, (dy, dx) in enumerate(
                            ((0, 0), (0, 1), (1, 0), (1, 1))):
                        for n in range(NT):
                            for q in range(2):
                                a = 2 * n + q + GB - dy
                                nc.sync.dma_start(
                                    v4[a, GB - dx:GB + W - dx, j, :],
                                    xt_sb[q * 64:(q + 1) * 64,
                                          n * 128 + g * Cg:
                                          n * 128 + (g + 1) * Cg])

            for g in range(DG):
                t_x = tap(f"xpr{g}", [NR, 4 * Cg], BF16)
                if t_x is not None:
                    nc.sync.dma_start(t_x[:, :], xprs[g][:, :])
            t_by = tap("by", [128, NT * NJ])
            if t_by is not None:
                nc.sync.dma_start(t_by[:, :], by_sb[:, :])
            t_bx = tap("bx", [128, NT * NJ])
            if t_bx is not None:
                nc.sync.dma_start(t_bx[:, :], bx_sb[:, :])
            t_xp = tap("xp", [C, HP * HP], BF16)
            if t_xp is not None:
                nc.sync.dma_start(t_xp[:, :], xp_sb[:, :])

            # ---- Phase B: offset conv -> om_sb [54, 4096] ----------------
            with tc.tile_pool(name="psBC", bufs=2, space="PSUM") as psBC:
                xp3 = xp_sb.rearrange("c (r q) -> c r q", q=HP)
                for ch in range(8):  # 8 chunks of 512 output pixels
                    om_ps = psBC.tile([NO, 512], F32, tag="omps", name="omps")
                    for k in range(KK):
                        ky, kx = k // KS, k % KS
                        rhs = xp3[:, 2 * ky + ch * 8: 2 * ky + ch * 8 + 8,
                                  2 * kx: 2 * kx + W]
                        nc.tensor.matmul(
                            om_ps[:, :], ow_sb[:, k, :], rhs,
                            start=(k == 0), stop=(k == KK - 1),
                        )
                    nc.scalar.activation(
                        om_sb[:, ch * 512:(ch + 1) * 512], om_ps[:, :],
                        ACTF.Identity, bias=ob_sb[:, :], scale=1.0,
                    )

                # ---- Phase C: transpose om -> omT [128, 32*54] -----------
                for n in range(NT):
                    tp = psBC.tile([128, NO], F32, tag="omt", name="omt")
                    nc.tensor.transpose(
                        tp[:, :], om_sb[:, n * 128:(n + 1) * 128],
                        ident[:NO, :NO]
                    )
                    nc.scalar.copy(omT[:, n * NO:(n + 1) * NO], tp[:, :])

            # ---- Phase D: coordinates, coefficients, indices --------------
            omT3 = omT.rearrange("p (n c) -> p n c", c=NO)
            offv = omT3[:, :, 0:NOFF].rearrange("p n (g k t) -> p n g k t",
                                                g=DG, k=KK)
            maskv = omT3[:, :, NOFF:NO].rearrange("p n (g k) -> p n g k", g=DG)

            def F(nm):
                return fpool.tile([128, NT * NJ], F32, name=nm)

            def v4f(t):  # [128, 576] -> [p, n, g, k] view (j-major layout)
                return t.rearrange("p (g k n) -> p n g k", g=DG, k=KK)

            py, px = F("py"), F("px")
            nc.vector.tensor_tensor(out=v4f(py), in0=offv[:, :, :, :, 0],
                                    in1=v4f(by_sb), op=AL.add)
            nc.vector.tensor_tensor(out=v4f(px), in0=offv[:, :, :, :, 1],
                                    in1=v4f(bx_sb), op=AL.add)
            for t_ in (py, px):
                nc.vector.tensor_scalar_max(t_[:, :], t_[:, :], -5.5)
                nc.vector.tensor_scalar_min(t_[:, :], t_[:, :], 67.5)

            def floor_of(src, nm):
                fl = F("fl_" + nm)
                ii = fpool.tile([128, NT * NJ], I32, name="ii_" + nm)
                nc.vector.tensor_scalar_add(fl[:, :], src[:, :], 1024.0)
                nc.vector.tensor_copy(out=ii[:, :], in_=fl[:, :])
                nc.vector.tensor_copy(out=fl[:, :], in_=ii[:, :])
                nc.vector.tensor_scalar_sub(fl[:, :], fl[:, :], 1024.0)
                fix = F("fix_" + nm)
                nc.vector.tensor_tensor(out=fix[:, :], in0=fl[:, :],
                                        in1=src[:, :], op=AL.is_gt)
                nc.vector.tensor_tensor(out=fl[:, :], in0=fl[:, :],
                                        in1=fix[:, :], op=AL.subtract)
                return fl

            y0, x0 = floor_of(py, "y"), floor_of(px, "x")
            wy, wx = F("wy"), F("wx")
            nc.vector.tensor_tensor(out=wy[:, :], in0=py[:, :], in1=y0[:, :],
                                    op=AL.subtract)
            nc.vector.tensor_tensor(out=wx[:, :], in0=px[:, :], in1=x0[:, :],
                                    op=AL.subtract)

            mm = F("mm")
            nc.scalar.activation(v4f(mm), maskv, ACTF.Sigmoid)
            nc.vector.tensor_scalar_mul(mm[:, :], mm[:, :], 2.0)

            beta, alpha = F("beta"), F("alpha")
            nc.vector.tensor_tensor(out=beta[:, :], in0=mm[:, :], in1=wy[:, :],
                                    op=AL.mult)
            nc.vector.tensor_tensor(out=alpha[:, :], in0=mm[:, :],
                                    in1=beta[:, :], op=AL.subtract)
            c01, c00 = F("c01"), F("c00")
            nc.vector.tensor_tensor(out=c01[:, :], in0=alpha[:, :],
                                    in1=wx[:, :], op=AL.mult)
            nc.vector.tensor_tensor(out=c00[:, :], in0=alpha[:, :],
                                    in1=c01[:, :], op=AL.subtract)
            c11, c10 = F("c11"), F("c10")
            nc.vector.tensor_tensor(out=c11[:, :], in0=beta[:, :],
                                    in1=wx[:, :], op=AL.mult)
            nc.vector.tensor_tensor(out=c10[:, :], in0=beta[:, :],
                                    in1=c11[:, :], op=AL.subtract)

            itf = F("itf")
            nc.vector.tensor_scalar(itf[:, :], y0[:, :], float(GY),
                                    float(IDX_OFF), AL.mult, AL.add)
            nc.vector.tensor_tensor(out=itf[:, :], in0=itf[:, :],
                                    in1=x0[:, :], op=AL.add)
            it_i = fpool.tile([128, NT * NJ], I32, name="it_i")
            nc.vector.tensor_copy(out=it_i[:, :], in_=itf[:, :])
            coefs = [c00, c01, c10, c11]
            for nm_, t_ in (("om", om_sb), ("py", py), ("px", px),
                            ("c00", c00), ("c01", c01), ("c10", c10),
                            ("c11", c11)):
                tt = tap(nm_, list(t_.shape))
                if tt is not None:
                    nc.sync.dma_start(tt[:, :], t_[:, :])
            t_it = tap("it", [128, NT * NJ], I32)
            if t_it is not None:
                nc.sync.dma_start(t_it[:, :], it_i[:, :])

            # ---- Phase E/F: gather, blend, transpose, main matmul ---------
            from contextlib import ExitStack
            ectx = ExitStack()
            gpool = ectx.enter_context(tc.tile_pool(name="gather", bufs=3))
            vpool = ectx.enter_context(tc.tile_pool(name="vpairp", bufs=2))
            vtpool = ectx.enter_context(tc.tile_pool(name="valtp", bufs=2))
            opool = ectx.enter_context(tc.tile_pool(name="outsbp", bufs=2))
            psO = ectx.enter_context(tc.tile_pool(name="psO", bufs=1,
                                                  space="PSUM"))
            psT = ectx.enter_context(tc.tile_pool(name="psT", bufs=4,
                                                  space="PSUM"))
            for half in range(2):
                out_ps = psO.tile([128, 2048], F32, tag="out", name="out_ps")
                n0 = half * 16
                for k in range(KK):
                    vpair = vpool.tile([128, 16, 128], F32, tag="vp",
                                       name="vpair")
                    for g in range(DG):
                        j = g * KK + k
                        gt = gpool.tile([128, 16, 256], BF16, tag="gt",
                                        name="gt")
                        for n in range(16):
                            ic = j * NT + n0 + n
                            nc.gpsimd.indirect_dma_start(
                                out=gt[:, n, :],
                                out_offset=None,
                                in_=xprs[g][:, :],
                                in_offset=bass.IndirectOffsetOnAxis(
                                    ap=it_i[:, ic:ic + 1], axis=0,
                                ),
                            )
                        if half == 0 and k == 0 and g == 0:
                            t_gt = tap("gt00", [128, 16, 256], BF16)
                            if t_gt is not None:
                                nc.sync.dma_start(t_gt[:, :, :], gt[:, :, :])
                        for n in range(16):
                            col = j * NT + (n0 + n)
                            vslice = vpair[:, n, g * Cg:(g + 1) * Cg]
                            srcs = (gt[:, n, 0:Cg], gt[:, n, Cg:2 * Cg],
                                    gt[:, n, 2 * Cg:3 * Cg],
                                    gt[:, n, 3 * Cg:4 * Cg])
                            nc.vector.tensor_scalar_mul(
                                vslice, srcs[0], coefs[0][:, col:col + 1])
                            for ci in range(1, 4):
                                nc.vector.scalar_tensor_tensor(
                                    out=vslice, in0=srcs[ci],
                                    scalar=coefs[ci][:, col:col + 1],
                                    in1=vslice, op0=AL.mult, op1=AL.add)
                    valT = vtpool.tile([128, 2048], BF16, tag="vt", name="valT")
                    for n in range(16):
                        tp = psT.tile([128, 128], F32, tag="vtp", name="tp_v")
                        nc.tensor.transpose(tp[:, :], vpair[:, n, :],
                                            ident[:, :])
                        nc.scalar.copy(valT[:, n * 128:(n + 1) * 128], tp[:, :])
                    for jc in range(4):
                        cs = slice(jc * 512, (jc + 1) * 512)
                        nc.tensor.matmul(
                            out_ps[:, cs], wm_sb[:, k, :], valT[:, cs],
                            start=(k == 0), stop=(k == KK - 1),
                        )
                o_sb = opool.tile([128, 2048], BF16, tag="osb", name="o_sb")
                for jc in range(4):
                    cs = slice(jc * 512, (jc + 1) * 512)
                    nc.scalar.copy(o_sb[:, cs], out_ps[:, cs])
                nc.sync.dma_start(out[:, half * 2048:(half + 1) * 2048],
                                  o_sb[:, :])
            ectx.close()
    nc.finalize()
    return nc


def host_inputs(x, offset_w, offset_b, weight):
    """Build the per-core input maps (core b <- batch element b)."""
    x = np.asarray(x, np.float32)
    offset_w = np.asarray(offset_w, np.float32)
    offset_b = np.asarray(offset_b, np.float32)
    weight = np.asarray(weight, np.float32)

    # Tap weights, block-diagonal over conv groups: [KK, C, NO]
    offw = np.zeros((KK, C, NO), np.float32)
    for k in range(KK):
        ky, kx = k // KS, k % KS
        for g in range(DG):
            # conv group g: out chans [g*27,(g+1)*27) <- in chans [g*64,(g+1)*64)
            offw[k, g * Cg:(g + 1) * Cg, g * 27:(g + 1) * 27] = \
                offset_w[g * 27:(g + 1) * 27, :, ky, kx].T
    offb = offset_b.reshape(NO, 1).copy()

    # Main weights: [KK, C, Co] with rows (g*64+c) = weight[o, g*64+c, ky, kx]
    wmain = np.zeros((KK, C, Co), np.float32)
    for k in range(KK):
        ky, kx = k // KS, k % KS
        wmain[k] = weight[:, :, ky, kx].T  # [C, Co]

    offw_bf = offw.astype(NPBF)
    wmain_bf = wmain.astype(NPBF)
    in_maps = []
    for b in range(B):
        in_maps.append({
            "x_in": x[b].reshape(C, S).astype(NPBF),
            "offw": offw_bf, "offb": offb, "wmain": wmain_bf,
        })
    return in_maps


_NC_CACHE = {}


def get_nc():
    if "nc" not in _NC_CACHE:
        _NC_CACHE["nc"] = build_nc()
    return _NC_CACHE["nc"]


def kernel(x, offset_w, offset_b, weight):
    nc = get_nc()
    in_maps = host_inputs(x, offset_w, offset_b, weight)
    res = run_bass_kernel_spmd(nc, in_maps, list(range(B)))
    outs = [res.results[b]["out"].astype(np.float32).reshape(Co, H, W)
            for b in range(B)]
    return np.stack(outs)
